# revision 20
# baseline (speedup 1.0000x reference)
"""AttentionPooling (ragged graph cross-attention pooling) on 8 TRN2 NeuronCores.

Strategy (SPMD, no collectives):
  * Host assigns 8 whole graphs to each of the 8 cores (serpentine by size),
    sorts each core's graphs by size into 8 "slots".  Slot j has a fixed tile
    count T[j] (shared by all cores, since the instruction stream is shared);
    each graph's edges are placed at its slot offset and zero-padded.
  * Host ships x^T (transposed edge features, bf16) per core + replicated
    weights.  Padding edges give exp(0)=1 in the softmax denominator, which is
    corrected with a host-computed per-slot pad count.
  * Softmax is computed without max-subtraction (scores ~ N(0,1); exp cannot
    overflow fp32) — mathematically identical to the reference's stable form.
  * Scores are linear in x: scores = (x @ w_k) . q  =  x @ Ws where
    Ws[:, (h,s)] = sum_d w_k[:, (h,d)] q[s,h,d] / sqrt(hd).  Ws ([256, 256])
    is host-precomputed from the weights and shipped fused with w_v as one
    [256, 512] operand, so the per-tile device work is:
      [v | sc][e, :] = x @ [w_v | Ws]    (PE, 2 matmuls/tile, N=512)
      ex             = exp(sc)           (ACT, psum->sbuf bf16)
      pooled[(h,s),(h,d)|denom] += ex.T @ [v | 1]  (PE, psum-accum per graph)
  * Per graph: denom -= npad; normalize by 1/denom (DVE); 32x32 block
    transpose (DVE StreamTranspose) to build the [128, (s,half)*8graphs]
    operand P2 for the MLP (w1 needs no permutation in this layout).
  * MLP: h1 = silu(pooled @ w1 + b1) (PE, 4-way tile_position-packed, +ACT),
    out = h1 @ w2 + b2 (PE), emitted as out^T [256, 8] per core; the host
    scatters core outputs into the final [64, 256].
"""

import os
import sys
from contextlib import ExitStack

import numpy as np

for _p in ("/opt/trn_rl_repo",):
    if _p not in sys.path:
        sys.path.append(_p)

import ml_dtypes  # noqa: E402

import concourse.bass as bass  # noqa: E402
import concourse.tile as tile  # noqa: E402
from concourse import mybir  # noqa: E402
from concourse.bass_utils import run_bass_kernel_spmd  # noqa: E402
from concourse.vector_clock import ScopedClock  # noqa: E402

BF16 = ml_dtypes.bfloat16

E, B, H, S, NH, HD = 131072, 64, 256, 32, 8, 32
NCORES = 8
NG = B // NCORES        # graphs (slots) per core
TILE = 128              # edge tile
GTILES = 4              # tiles per xt DMA group
GROUP = GTILES * TILE   # edges per group
SCALE = 1.0 / float(np.sqrt(HD))

AF = mybir.ActivationFunctionType

# ---------------------------------------------------------------------------
# Walrus workaround: this toolchain's InstDrain accepts only ONE sync wait;
# Tile's kernel-tail drain carries one wait per outstanding semaphore.
# Split it into a chain of single-wait drains.
_MAXW = 1


def _split_drain_and_barrier(self, tick_clock, wait_clock):
    nc = self.nc
    drain_inst = nc.sync.drain()
    wait_clock.add_sem_waits(
        drain_inst.ins, ScopedClock({None: tick_clock.global_clock})
    )
    waits = list(drain_inst.ins.sync_info.on_wait)
    if len(waits) > _MAXW:
        drain_inst.ins.sync_info = mybir.SyncInfo(on_wait=waits[:_MAXW], on_update=[])
        for i in range(_MAXW, len(waits), _MAXW):
            d2 = nc.sync.drain()
            d2.ins.sync_info = mybir.SyncInfo(
                on_wait=waits[i : i + _MAXW], on_update=[]
            )
    nc.all_engine_barrier()
    popped = nc._tile_sem_poison_stack.pop()
    assert popped is self._sem_poison
    nc.clear_and_free_semaphores(list(self.sems.allocated().values()))
    nc.all_engine_barrier()


tile.TileContext._drain_and_barrier = _split_drain_and_barrier

# Engine instructions are capped at 2 sync waits by this walrus (Drain/NoOp
# at 1).  Tile's sem-assignment occasionally emits more.  Hoist the excess
# onto single-wait NoOps inserted just before, on the same engine — the
# engine stalls at the NoOp instead, which is semantically identical.
_WAIT_CAP = {"InstDrain": 1}
_WAIT_CAP_DEFAULT = 1


def _fix_excess_waits(nc):
    n_fixed = 0
    for fn in nc.m.functions:
        for bb in fn.blocks:
            insts = bb.instructions
            out = []
            changed = False
            for inst in insts:
                si = inst.sync_info
                waits = list(si.on_wait) if si is not None else []
                cap = _WAIT_CAP.get(type(inst).__name__, _WAIT_CAP_DEFAULT)
                if len(waits) > cap:
                    changed = True
                    n_fixed += 1
                    excess = waits[: len(waits) - cap]
                    for i, w in enumerate(excess):
                        nop = mybir.InstNoOp(
                            name=f"{inst.name}-hw{i}", ins=[], outs=[]
                        )
                        nop.engine = inst.engine
                        nop.sync_info = mybir.SyncInfo(on_wait=[w], on_update=[])
                        out.append(nop)
                    inst.sync_info = mybir.SyncInfo(
                        on_wait=waits[len(excess) :], on_update=list(si.on_update)
                    )
                out.append(inst)
            if changed:
                bb.instructions = out
    return n_fixed

# ---------------------------------------------------------------------------

_PROGRAM_CACHE: dict[tuple, "bass.Bass"] = {}
LAST_RESULTS = None  # BassKernelResults of the most recent run (for testing)


def _install_ntff_hook_shim():
    """The image's antenv lacks axon_hooks; recreate it so trace=True works."""
    try:
        import types

        import antenv

        if "antenv.axon_hooks" not in sys.modules:
            mod = types.ModuleType("antenv.axon_hooks")
            mod._hook = None

            def set_axon_ntff_profile_hook(h):
                mod._hook = h

            def get_axon_ntff_profile_hook():
                return mod._hook

            mod.set_axon_ntff_profile_hook = set_axon_ntff_profile_hook
            mod.get_axon_ntff_profile_hook = get_axon_ntff_profile_hook
            sys.modules["antenv.axon_hooks"] = mod
            antenv.axon_hooks = mod
        import antenv.axon_hooks as ah

        if ah.get_axon_ntff_profile_hook() is None:
            from trn_agent_boot.trn_boot import _ntff_profile_via_ctypes

            ah.set_axon_ntff_profile_hook(
                _ntff_profile_via_ctypes("/opt/axon/libaxon_pjrt.so")
            )
    except Exception:
        pass


_install_ntff_hook_shim()

# Optional experiment: let walrus double-buffer LDWEIGHTS (default off here).
import concourse.bass_utils as _bass_utils  # noqa: E402

_orig_run_command = _bass_utils.run_command


def _run_command_ldwopt(cmd, **kw):
    if isinstance(cmd, list):
        cmd = [
            "--enable-ldw-opt=true" if c == "--enable-ldw-opt=false" else c
            for c in cmd
        ]
    return _orig_run_command(cmd, **kw)


if os.environ.get("KERNEL_LDW_OPT") == "1":
    _bass_utils.run_command = _run_command_ldwopt


def build_program(slot_tiles: tuple[int, ...]) -> "bass.Bass":
    """Build the SPMD Bass program for per-core slot tile counts."""
    TT = sum(slot_tiles)
    EC = TT * TILE
    assert TT % GTILES == 0
    NGRP = TT // GTILES

    # per-tile slot id / first / last flags
    slot_of, first_of, last_of = [], [], []
    for j, tj in enumerate(slot_tiles):
        for t in range(tj):
            slot_of.append(j)
            first_of.append(t == 0)
            last_of.append(t == tj - 1)

    f32, bf16 = mybir.dt.float32, mybir.dt.bfloat16
    nc = bass.Bass("TRN2", target_bir_lowering=False, debug=False, num_devices=NCORES)

    # xt is k-interleaved on the host: column block [t*256:(t+1)*256] holds
    # edge-tile t's two contraction halves side by side, so one contiguous
    # DMA (2KB/partition-row) fetches a whole group of GTILES tiles.
    xt_d = nc.dram_tensor("xt", [128, 2 * EC], bf16, kind="ExternalInput").ap()
    wvs_d = nc.dram_tensor("wvs", [H, 2 * H], bf16, kind="ExternalInput").ap()
    w1_d = nc.dram_tensor("w1", [S * H, H], bf16, kind="ExternalInput").ap()
    w2_d = nc.dram_tensor("w2", [H, H], bf16, kind="ExternalInput").ap()
    b1_d = nc.dram_tensor("b1", [NG, H], f32, kind="ExternalInput").ap()
    b2_d = nc.dram_tensor("b2", [H, 1], f32, kind="ExternalInput").ap()
    npad_d = nc.dram_tensor("npad", [128, NG], f32, kind="ExternalInput").ap()
    ident_d = nc.dram_tensor("ident", [128, 128], bf16, kind="ExternalInput").ap()
    outT_d = nc.dram_tensor("outT", [H, NG], f32, kind="ExternalOutput").ap()

    with tile.TileContext(nc) as tc, ExitStack() as ctx:
        const = ctx.enter_context(tc.tile_pool(name="const", bufs=1))
        w2_sb = const.tile([128, 2 * H], bf16)
        wvs_sb = const.tile([128, 2 * 2 * H], bf16)  # k-tile k: [wv_k | ws_k]
        w1_sb = const.tile([128, 64 * H], bf16)
        ident_sb = const.tile([128, 128], bf16)
        b1_sb = const.tile([NG, H], f32)
        b2_sb = const.tile([128, 2], f32)
        npad_sb = const.tile([128, NG], f32)
        P2 = const.tile([128, 64 * NG], bf16)

        for k in range(2):
            r = slice(k * 128, (k + 1) * 128)
            nc.scalar.dma_start(wvs_sb[:, k * 2 * H : (k + 1) * 2 * H], wvs_d[r, :])
        nc.scalar.dma_start(npad_sb[:], npad_d[:])
        for k in range(2):
            r = slice(k * 128, (k + 1) * 128)
            nc.scalar.dma_start(w2_sb[:, k * H : (k + 1) * H], w2_d[r, :])
            nc.scalar.dma_start(b2_sb[:, k : k + 1], b2_d[r, :])
        nc.scalar.dma_start(ident_sb[:], ident_d[:])
        nc.scalar.dma_start(b1_sb[:], b1_d[:])

        # Warm the Exp ACT table while the first DMAs are in flight, so the
        # table load is off the critical path.  (ACT holds one table at a
        # time — warming Silu here would evict Exp and cost a reload at the
        # loop head; the single Silu load in the MLP tail hides behind MLP1.)
        warm = const.tile([1, 2], f32)
        nc.gpsimd.memset(warm[:, 0:1], 0.0)
        nc.scalar.activation(warm[:, 1:2], warm[:, 0:1], AF.Exp)

        # ---- main edge loop ---------------------------------------------
        xt_pool = ctx.enter_context(tc.tile_pool(name="xtp", bufs=6))
        ex_pool = ctx.enter_context(tc.tile_pool(name="exp", bufs=6))
        ext_pool = ctx.enter_context(tc.tile_pool(name="ext", bufs=2))

        NRING = 6
        vs_ring = [const.tile([128, 258], bf16, name=f"vsring{i}") for i in range(NRING)]
        for t in vs_ring:
            nc.vector.memset(t[:, 128:129], 1.0)
            nc.vector.memset(t[:, 257:258], 1.0)

        pooled_tiles: list = [None, None]

        def emit_pooled(sl, fi, la, ex, vs):
            if fi:
                pooled_tiles[0] = pl_pool.tile([128, 129], f32, tag="pl0", name=f"pl0_s{sl}")
                pooled_tiles[1] = pl_pool.tile([128, 129], f32, tag="pl1", name=f"pl1_s{sl}")
            for m in range(2):
                nc.tensor.matmul(
                    pooled_tiles[m][:],
                    ex[:, m * 128 : (m + 1) * 128],
                    vs[:, m * 129 : m * 129 + 129],
                    start=fi,
                    stop=la,
                )
            if la:
                extract_graph(sl, pooled_tiles)

        P2v = P2[:].rearrange("p (s x) -> p s x", x=2 * NG)

        def extract_graph(g, ptiles):
            last = g == NG - 1
            for m in range(2):
                # For the last graph the scatter is on the MLP critical path:
                # split it across DVE and GpSimd so the halves run in parallel.
                copy_eng = nc.gpsimd if (not last or m == 1) else nc.vector
                den = ext_pool.tile([128, 1], f32, tag="den", name=f"den{g}_{m}")
                nc.vector.tensor_scalar_sub(
                    den[:], ptiles[m][:, 128:129], npad_sb[:, g : g + 1]
                )
                rec = ext_pool.tile([128, 1], f32, tag="rec", name=f"rec{g}_{m}")
                nc.vector.reciprocal(rec[:], den[:])
                pn = ext_pool.tile([128, 128], f32, tag="pn", name=f"pn{g}_{m}")
                nc.vector.tensor_scalar_mul(pn[:], ptiles[m][:, 0:128], rec[:])
                pt = ext_pool.tile([128, 128], f32, tag="pt", name=f"pt{g}_{m}")
                nc.vector.transpose(pt[:], pn[:])
                for hh in range(4):
                    rr = slice(hh * 32, (hh + 1) * 32)
                    src = pt[rr, hh * 32 : (hh + 1) * 32].rearrange(
                        "p (a o) -> p a o", o=1
                    )
                    copy_eng.tensor_copy(P2v[rr, :, m * NG + g : m * NG + g + 1], src)

        with (
            tc.tile_pool(name="vscp", bufs=4, space="PSUM") as vsc_pool,
            tc.tile_pool(name="plp", bufs=2, space="PSUM") as pl_pool,
        ):
            from collections import deque

            pending = deque()
            tidx = 0
            for grp in range(NGRP):
                xt = xt_pool.tile([128, 2 * GROUP], bf16, tag="xt", name=f"xt_{grp}")
                nc.sync.dma_start(
                    xt[:], xt_d[:, grp * 2 * GROUP : (grp + 1) * 2 * GROUP]
                )
                for sub in range(GTILES):
                    sl, fi, la = slot_of[tidx], first_of[tidx], last_of[tidx]
                    e0 = sub * 2 * TILE
                    vsc = vsc_pool.tile([128, 512], f32, tag="vsc", name=f"vsc{tidx}")
                    for k in range(2):
                        nc.tensor.matmul(
                            vsc[:],
                            xt[:, e0 + k * TILE : e0 + (k + 1) * TILE],
                            wvs_sb[:, k * 2 * H : (k + 1) * 2 * H],
                            start=(k == 0),
                            stop=(k == 1),
                        )
                    ex = ex_pool.tile([128, 256], bf16, tag="ex", name=f"ex{tidx}")
                    nc.scalar.activation(ex[:], vsc[:, H : 2 * H], AF.Exp)
                    vs = vs_ring[tidx % NRING]
                    nc.vector.tensor_copy(
                        vs[:].rearrange("p (b c) -> p b c", c=129)[:, :, 0:128],
                        vsc[:, 0:H].rearrange("p (b c) -> p b c", c=128),
                    )
                    pending.append((sl, fi, la, ex, vs))
                    while len(pending) > 2:
                        emit_pooled(*pending.popleft())
                    tidx += 1
            while pending:
                emit_pooled(*pending.popleft())

        # w1 load — one big blocked DMA on the Scalar HWDGE ring, so its 4MB
        # transfer cannot queue ahead of the edge-loop xt groups on the Sync
        # ring (the scheduler hoists it regardless of trace position).
        nc.scalar.dma_start(
            w1_sb[:].rearrange("p (k c) -> p k c", c=H),
            w1_d[:].rearrange("(k p) c -> p k c", p=128),
        )

        # ---- MLP tail ----------------------------------------------------
        with (
            tc.tile_pool(name="mlpp", bufs=2, space="PSUM") as mp,
            tc.tile_pool(name="mlps", bufs=2) as ms,
        ):
            h1pp = mp.tile([128, H], f32, tag="h1pp")
            for j in range(64):
                q = j % 4
                nc.tensor.matmul(
                    h1pp[q * 32 : q * 32 + NG, :],
                    P2[:, j * NG : (j + 1) * NG],
                    w1_sb[:, j * H : (j + 1) * H],
                    start=(j < 4),
                    stop=(j >= 60),
                    tile_position=(0, q * 32),
                    skip_group_check=True,
                )
            # Sum the 4 quadrant partials out of PSUM on DVE (one PSUM
            # operand per op; GpSimd cannot touch PSUM) — replaces the
            # serialized quadrant copies + qsel matmul.
            c0 = ms.tile([NG, H], f32, tag="c0")
            nc.vector.tensor_copy(c0[:], h1pp[0:NG, :])
            t1 = ms.tile([NG, H], f32, tag="t1")
            nc.vector.tensor_add(t1[:], h1pp[32 : 32 + NG, :], c0[:])
            t2 = ms.tile([NG, H], f32, tag="t2")
            nc.vector.tensor_add(t2[:], h1pp[64 : 64 + NG, :], t1[:])
            t3 = ms.tile([NG, H], f32, tag="t3")
            nc.vector.tensor_add(t3[:], h1pp[96 : 96 + NG, :], t2[:])
            h1s = ms.tile([NG, H], f32, tag="h1s")
            nc.vector.tensor_add(h1s[:], t3[:], b1_sb[:])
            h1b = ms.tile([NG, H], bf16, tag="h1b")
            nc.scalar.activation(h1b[:], h1s[:], AF.Silu)
            h1t = []
            for m in range(2):
                h1tp = mp.tile([128, NG], bf16, tag="h1tp", name=f"h1tp{m}")
                nc.tensor.transpose(
                    h1tp[:], h1b[:, m * 128 : (m + 1) * 128], ident_sb[0:NG, 0:NG]
                )
                ht = ms.tile([128, NG], bf16, tag=f"h1t{m}")
                nc.vector.tensor_copy(ht[:], h1tp[:])
                h1t.append(ht)
            osb = ms.tile([128, 2 * NG], f32, tag="osb")
            for m in range(2):
                otp = mp.tile([128, NG], f32, tag="otp", name=f"otp{m}")
                for k in range(2):
                    nc.tensor.matmul(
                        otp[:],
                        w2_sb[:, k * H + m * 128 : k * H + m * 128 + 128],
                        h1t[k][:],
                        start=(k == 0),
                        stop=(k == 1),
                    )
                nc.vector.tensor_scalar_add(
                    osb[:, m * NG : (m + 1) * NG], otp[:], b2_sb[:, m : m + 1]
                )
            nc.sync.dma_start(
                outT_d[:].rearrange("(m p) g -> p m g", m=2),
                osb[:].rearrange("p (m g) -> p m g", m=2),
            )

    return nc


def get_program(slot_tiles: tuple[int, ...]) -> "bass.Bass":
    if slot_tiles not in _PROGRAM_CACHE:
        nc = build_program(slot_tiles)
        # HW-path only (CoreSim snapshots the program before this pass)
        _fix_excess_waits(nc)
        _PROGRAM_CACHE[slot_tiles] = nc
    return _PROGRAM_CACHE[slot_tiles]


# ---------------------------------------------------------------------------
# Host-side sharding / padding


def plan_shards(batch: np.ndarray):
    """Returns (assign [NCORES][NG] graph ids, slot_tiles tuple, sizes)."""
    sizes = np.bincount(batch, minlength=B).astype(np.int64)
    order = np.argsort(-sizes, kind="stable")
    assign = [[] for _ in range(NCORES)]
    for r in range(NG):
        row = order[r * NCORES : (r + 1) * NCORES]
        if r % 2 == 1:
            row = row[::-1]
        for c in range(NCORES):
            assign[c].append(int(row[c]))
    for c in range(NCORES):
        assign[c].sort(key=lambda g: -sizes[g])
    slot_tiles = []
    for j in range(NG):
        mx = max(sizes[assign[c][j]] for c in range(NCORES))
        slot_tiles.append(int(max(1, -(-mx // TILE))))
    # round total tiles up to a GROUP multiple (pad goes to the last slot)
    rem = (-sum(slot_tiles)) % (GROUP // TILE)
    slot_tiles[-1] += rem
    return assign, tuple(slot_tiles), sizes


def make_in_maps(edge_features, batch, seed_vectors, w_q, w_k, w_v, w1, b1, w2, b2):
    edge_features = np.asarray(edge_features, dtype=np.float32)
    batch = np.asarray(batch)
    assign, slot_tiles, sizes = plan_shards(batch)
    TT = sum(slot_tiles)
    EC = TT * TILE

    starts = np.searchsorted(batch, np.arange(B))
    xb = edge_features.astype(BF16)

    # Ws[hin, h*S+s] = sum_d w_k[hin, h*HD+d] * q[s, h, d] / sqrt(HD)
    q = (np.asarray(seed_vectors, np.float32) @ np.asarray(w_q, np.float32)).reshape(
        S, NH, HD
    )
    wk3 = np.asarray(w_k, np.float32).reshape(H, NH, HD)
    Ws = (np.einsum("ihd,shd->ihs", wk3, q) * SCALE).reshape(H, NH * S)
    wvs = np.concatenate([np.asarray(w_v, np.float32), Ws], axis=1)

    shared = {
        "wvs": np.ascontiguousarray(wvs.astype(BF16)),
        "w1": np.ascontiguousarray(np.asarray(w1).astype(BF16)),
        "w2": np.ascontiguousarray(np.asarray(w2).astype(BF16)),
        "b1": np.ascontiguousarray(
            np.broadcast_to(np.asarray(b1, dtype=np.float32), (NG, H))
        ),
        "b2": np.ascontiguousarray(np.asarray(b2, dtype=np.float32).reshape(H, 1)),
        "ident": np.eye(128, dtype=BF16),
    }

    in_maps = []
    for c in range(NCORES):
        xt = np.zeros((H, EC), dtype=BF16)
        npad = np.zeros(NG, dtype=np.float32)
        off = 0
        for j, g in enumerate(assign[c]):
            n = int(sizes[g])
            xt[:, off : off + n] = xb[starts[g] : starts[g] + n].T
            npad[j] = slot_tiles[j] * TILE - n
            off += slot_tiles[j] * TILE
        # k-interleave: [2, 128, TT, 128] (k, p, t, c) -> [128, TT, 2, 128]
        xti = np.ascontiguousarray(
            xt.reshape(2, 128, TT, TILE).transpose(1, 2, 0, 3).reshape(128, 2 * EC)
        )
        m = dict(shared)
        m["xt"] = xti
        m["npad"] = np.ascontiguousarray(np.broadcast_to(npad, (128, NG)))
        in_maps.append(m)
    return in_maps, assign, slot_tiles


def kernel(
    edge_features,
    edge_coords,
    batch,
    seed_vectors,
    w_q,
    w_k,
    w_v,
    w1,
    b1,
    w2,
    b2,
):
    in_maps, assign, slot_tiles = make_in_maps(
        edge_features, batch, seed_vectors, w_q, w_k, w_v, w1, b1, w2, b2
    )
    nc = get_program(slot_tiles)

    res = run_bass_kernel_spmd(nc, in_maps, core_ids=list(range(NCORES)))
    global LAST_RESULTS
    LAST_RESULTS = res

    out = np.zeros((B, H), dtype=np.float32)
    for c in range(NCORES):
        outT = res.results[c]["outT"]  # [H, NG]
        for j, g in enumerate(assign[c]):
            out[g, :] = outT[:, j]
    return out



# revision 33
# speedup vs baseline: 1.1728x; 1.1728x over previous
"""AttentionPooling (ragged graph cross-attention pooling) on 8 TRN2 NeuronCores.

Strategy (SPMD, no collectives):
  * Host assigns 8 whole graphs to each of the 8 cores (serpentine by size),
    sorts each core's graphs by size into 8 "slots".  Slot j has a fixed tile
    count T[j] (shared by all cores, since the instruction stream is shared);
    each graph's edges are placed at its slot offset and zero-padded.
  * Host ships x^T (transposed edge features, bf16) per core + replicated
    weights.  Padding edges give exp(0)=1 in the softmax denominator, which is
    corrected with a host-computed per-slot pad count.
  * Softmax is computed without max-subtraction (scores ~ N(0,1); exp cannot
    overflow fp32) — mathematically identical to the reference's stable form.
  * Scores are linear in x: scores = (x @ w_k) . q  =  x @ Ws where
    Ws[:, (h,s)] = sum_d w_k[:, (h,d)] q[s,h,d] / sqrt(hd).  Ws ([256, 256])
    is host-precomputed from the weights and shipped fused with w_v as one
    [256, 512] operand, so the per-tile device work is:
      [v | sc][e, :] = x @ [w_v | Ws]    (PE, 2 matmuls/tile, N=512)
      ex             = exp(sc)           (ACT, psum->sbuf bf16)
      pooled[(h,s),(h,d)|denom] += ex.T @ [v | 1]  (PE, psum-accum per graph)
  * Per graph: denom -= npad; normalize by 1/denom (DVE); 32x32 block
    transpose (DVE StreamTranspose) to build the [128, (s,half)*8graphs]
    operand P2 for the MLP (w1 needs no permutation in this layout).
  * MLP: h1 = silu(pooled @ w1 + b1) (PE, 4-way tile_position-packed, +ACT),
    out = h1 @ w2 + b2 (PE), emitted as out^T [256, 8] per core; the host
    scatters core outputs into the final [64, 256].
"""

import os
import sys
from contextlib import ExitStack

import numpy as np

for _p in ("/opt/trn_rl_repo",):
    if _p not in sys.path:
        sys.path.append(_p)

import ml_dtypes  # noqa: E402

import concourse.bass as bass  # noqa: E402
import concourse.tile as tile  # noqa: E402
from concourse import mybir  # noqa: E402
from concourse.bass_utils import run_bass_kernel_spmd  # noqa: E402
from concourse.vector_clock import ScopedClock  # noqa: E402

BF16 = ml_dtypes.bfloat16

E, B, H, S, NH, HD = 131072, 64, 256, 32, 8, 32
NCORES = 8
NG = B // NCORES        # graphs (slots) per core
TILE = 128              # edge tile
GTILES = 4              # tiles per xt DMA group
GROUP = GTILES * TILE   # edges per group
SCALE = 1.0 / float(np.sqrt(HD))

AF = mybir.ActivationFunctionType

# ---------------------------------------------------------------------------
# Walrus workaround: this toolchain's InstDrain accepts only ONE sync wait;
# Tile's kernel-tail drain carries one wait per outstanding semaphore.
# Split it into a chain of single-wait drains.
_MAXW = 1


def _split_drain_and_barrier(self, tick_clock, wait_clock):
    nc = self.nc
    drain_inst = nc.sync.drain()
    wait_clock.add_sem_waits(
        drain_inst.ins, ScopedClock({None: tick_clock.global_clock})
    )
    waits = list(drain_inst.ins.sync_info.on_wait)
    if len(waits) > _MAXW:
        drain_inst.ins.sync_info = mybir.SyncInfo(on_wait=waits[:_MAXW], on_update=[])
        for i in range(_MAXW, len(waits), _MAXW):
            d2 = nc.sync.drain()
            d2.ins.sync_info = mybir.SyncInfo(
                on_wait=waits[i : i + _MAXW], on_update=[]
            )
    nc.all_engine_barrier()
    popped = nc._tile_sem_poison_stack.pop()
    assert popped is self._sem_poison
    nc.clear_and_free_semaphores(list(self.sems.allocated().values()))
    nc.all_engine_barrier()


tile.TileContext._drain_and_barrier = _split_drain_and_barrier

# Engine instructions are capped at 2 sync waits by this walrus (Drain/NoOp
# at 1).  Tile's sem-assignment occasionally emits more.  Hoist the excess
# onto single-wait NoOps inserted just before, on the same engine — the
# engine stalls at the NoOp instead, which is semantically identical.
_WAIT_CAP = {"InstDrain": 1}
_WAIT_CAP_DEFAULT = 1


def _fix_excess_waits(nc):
    n_fixed = 0
    for fn in nc.m.functions:
        for bb in fn.blocks:
            insts = bb.instructions
            out = []
            changed = False
            for inst in insts:
                si = inst.sync_info
                waits = list(si.on_wait) if si is not None else []
                cap = _WAIT_CAP.get(type(inst).__name__, _WAIT_CAP_DEFAULT)
                if len(waits) > cap:
                    changed = True
                    n_fixed += 1
                    excess = waits[: len(waits) - cap]
                    for i, w in enumerate(excess):
                        nop = mybir.InstNoOp(
                            name=f"{inst.name}-hw{i}", ins=[], outs=[]
                        )
                        nop.engine = inst.engine
                        nop.sync_info = mybir.SyncInfo(on_wait=[w], on_update=[])
                        out.append(nop)
                    inst.sync_info = mybir.SyncInfo(
                        on_wait=waits[len(excess) :], on_update=list(si.on_update)
                    )
                out.append(inst)
            if changed:
                bb.instructions = out
    return n_fixed

# ---------------------------------------------------------------------------

_PROGRAM_CACHE: dict[tuple, "bass.Bass"] = {}
LAST_RESULTS = None  # BassKernelResults of the most recent run (for testing)


def _install_ntff_hook_shim():
    """The image's antenv lacks axon_hooks; recreate it so trace=True works."""
    try:
        import types

        import antenv

        if "antenv.axon_hooks" not in sys.modules:
            mod = types.ModuleType("antenv.axon_hooks")
            mod._hook = None

            def set_axon_ntff_profile_hook(h):
                mod._hook = h

            def get_axon_ntff_profile_hook():
                return mod._hook

            mod.set_axon_ntff_profile_hook = set_axon_ntff_profile_hook
            mod.get_axon_ntff_profile_hook = get_axon_ntff_profile_hook
            sys.modules["antenv.axon_hooks"] = mod
            antenv.axon_hooks = mod
        import antenv.axon_hooks as ah

        if ah.get_axon_ntff_profile_hook() is None:
            from trn_agent_boot.trn_boot import _ntff_profile_via_ctypes

            ah.set_axon_ntff_profile_hook(
                _ntff_profile_via_ctypes("/opt/axon/libaxon_pjrt.so")
            )
    except Exception:
        pass


_install_ntff_hook_shim()

# Optional experiment: let walrus double-buffer LDWEIGHTS (default off here).
import concourse.bass_utils as _bass_utils  # noqa: E402

_orig_run_command = _bass_utils.run_command


def _run_command_ldwopt(cmd, **kw):
    if isinstance(cmd, list):
        cmd = [
            "--enable-ldw-opt=true" if c == "--enable-ldw-opt=false" else c
            for c in cmd
        ]
    return _orig_run_command(cmd, **kw)


if os.environ.get("KERNEL_LDW_OPT") == "1":
    _bass_utils.run_command = _run_command_ldwopt


def build_program(slot_tiles: tuple[int, ...]) -> "bass.Bass":
    """Build the SPMD Bass program for per-core slot tile counts."""
    TT = sum(slot_tiles)
    EC = TT * TILE
    assert TT % GTILES == 0
    NGRP = TT // GTILES

    # per-tile slot id / first / last flags
    slot_of, first_of, last_of = [], [], []
    for j, tj in enumerate(slot_tiles):
        for t in range(tj):
            slot_of.append(j)
            first_of.append(t == 0)
            last_of.append(t == tj - 1)

    f32, bf16 = mybir.dt.float32, mybir.dt.bfloat16
    nc = bass.Bass("TRN2", target_bir_lowering=False, debug=False, num_devices=NCORES)

    # xt is k-interleaved on the host: column block [t*256:(t+1)*256] holds
    # edge-tile t's two contraction halves side by side, so one contiguous
    # DMA (2KB/partition-row) fetches a whole group of GTILES tiles.
    xt_d = nc.dram_tensor("xt", [128, 2 * EC], bf16, kind="ExternalInput").ap()
    wvs_d = nc.dram_tensor("wvs", [H, 2 * H], bf16, kind="ExternalInput").ap()
    w1_d = nc.dram_tensor("w1", [S * H, H], bf16, kind="ExternalInput").ap()
    w2_d = nc.dram_tensor("w2", [H, H], bf16, kind="ExternalInput").ap()
    b1_d = nc.dram_tensor("b1", [128, 2], f32, kind="ExternalInput").ap()
    b2_d = nc.dram_tensor("b2", [H, 1], f32, kind="ExternalInput").ap()
    npad_d = nc.dram_tensor("npad", [128, NG], f32, kind="ExternalInput").ap()
    qsel_d = nc.dram_tensor("qsel", [128, NG], bf16, kind="ExternalInput").ap()
    outT_d = nc.dram_tensor("outT", [H, NG], f32, kind="ExternalOutput").ap()

    with tile.TileContext(nc) as tc, ExitStack() as ctx:
        const = ctx.enter_context(tc.tile_pool(name="const", bufs=1))
        w2_sb = const.tile([128, 2 * H], bf16)
        wvs_sb = const.tile([128, 2 * 2 * H], bf16)  # k-tile k: [wv_k | ws_k]
        w1_sb = const.tile([128, 64 * H], bf16)
        qsel_sb = const.tile([128, NG], bf16)
        b1_sb = const.tile([128, 2], f32)
        b2_sb = const.tile([128, 2], f32)
        npad_sb = const.tile([128, NG], f32)
        P2 = const.tile([128, 64 * NG], bf16)

        # DMA ring assignment matters: PSEUDO_DMA occupies the issuing
        # ENGINE for the transfer duration.  Keep the ACT (scalar) queue free
        # for the Exp table warm + per-tile exps: wvs rides the sync ring
        # just ahead of the xt stream; small late-use constants go on the
        # vector/gpsimd rings, which are idle at startup.
        for k in range(2):
            r = slice(k * 128, (k + 1) * 128)
            nc.sync.dma_start(wvs_sb[:, k * 2 * H : (k + 1) * 2 * H], wvs_d[r, :])
        nc.gpsimd.dma_start(npad_sb[:], npad_d[:])
        for k in range(2):
            r = slice(k * 128, (k + 1) * 128)
            nc.gpsimd.dma_start(w2_sb[:, k * H : (k + 1) * H], w2_d[r, :])
            nc.gpsimd.dma_start(b2_sb[:, k : k + 1], b2_d[r, :])
        nc.gpsimd.dma_start(b1_sb[:], b1_d[:])
        nc.gpsimd.dma_start(qsel_sb[:], qsel_d[:])

        # Warm the Exp ACT table while the first DMAs are in flight, so the
        # table load is off the critical path.  (ACT holds one table at a
        # time — warming Silu here would evict Exp and cost a reload at the
        # loop head; the single Silu load in the MLP tail hides behind MLP1.)
        warm = const.tile([1, 2], f32)
        nc.gpsimd.memset(warm[:, 0:1], 0.0)
        nc.scalar.activation(warm[:, 1:2], warm[:, 0:1], AF.Exp)

        # ---- main edge loop ---------------------------------------------
        xt_pool = ctx.enter_context(tc.tile_pool(name="xtp", bufs=6))
        ex_pool = ctx.enter_context(tc.tile_pool(name="exp", bufs=6))
        ext_pool = ctx.enter_context(tc.tile_pool(name="ext", bufs=2))

        NRING = 6
        vs_ring = [const.tile([128, 258], bf16, name=f"vsring{i}") for i in range(NRING)]
        for t in vs_ring:
            nc.vector.memset(t[:, 128:129], 1.0)
            nc.vector.memset(t[:, 257:258], 1.0)

        pooled_tiles: list = [None, None]

        def emit_pooled(sl, fi, la, ex, vs):
            if fi:
                pooled_tiles[0] = pl_pool.tile([128, 129], f32, tag="pl0", name=f"pl0_s{sl}")
                pooled_tiles[1] = pl_pool.tile([128, 129], f32, tag="pl1", name=f"pl1_s{sl}")
            for m in range(2):
                nc.tensor.matmul(
                    pooled_tiles[m][:],
                    ex[:, m * 128 : (m + 1) * 128],
                    vs[:, m * 129 : m * 129 + 129],
                    start=fi,
                    stop=la,
                )
            if la:
                extract_graph(sl, pooled_tiles)

        P2v = P2[:].rearrange("p (s x) -> p s x", x=2 * NG)

        def extract_graph(g, ptiles):
            last = g == NG - 1
            for m in range(2):
                # For the last graph the scatter is on the MLP critical path:
                # split it across DVE and GpSimd so the halves run in parallel.
                copy_eng = nc.gpsimd if (not last or m == 1) else nc.vector
                den = ext_pool.tile([128, 1], f32, tag="den", name=f"den{g}_{m}")
                nc.vector.tensor_scalar_sub(
                    den[:], ptiles[m][:, 128:129], npad_sb[:, g : g + 1]
                )
                rec = ext_pool.tile([128, 1], f32, tag="rec", name=f"rec{g}_{m}")
                nc.vector.reciprocal(rec[:], den[:])
                pn = ext_pool.tile([128, 128], f32, tag="pn", name=f"pn{g}_{m}")
                nc.vector.tensor_scalar_mul(pn[:], ptiles[m][:, 0:128], rec[:])
                pt = ext_pool.tile([128, 128], f32, tag="pt", name=f"pt{g}_{m}")
                nc.vector.transpose(pt[:], pn[:])
                for hh in range(4):
                    rr = slice(hh * 32, (hh + 1) * 32)
                    src = pt[rr, hh * 32 : (hh + 1) * 32].rearrange(
                        "p (a o) -> p a o", o=1
                    )
                    copy_eng.tensor_copy(P2v[rr, :, m * NG + g : m * NG + g + 1], src)

        with (
            tc.tile_pool(name="vscp", bufs=4, space="PSUM") as vsc_pool,
            tc.tile_pool(name="plp", bufs=2, space="PSUM") as pl_pool,
        ):
            from collections import deque

            pending = deque()
            tidx = 0
            for grp in range(NGRP):
                xt = xt_pool.tile([128, 2 * GROUP], bf16, tag="xt", name=f"xt_{grp}")
                nc.sync.dma_start(
                    xt[:], xt_d[:, grp * 2 * GROUP : (grp + 1) * 2 * GROUP]
                )
                for sub in range(GTILES):
                    sl, fi, la = slot_of[tidx], first_of[tidx], last_of[tidx]
                    e0 = sub * 2 * TILE
                    vsc = vsc_pool.tile([128, 512], f32, tag="vsc", name=f"vsc{tidx}")
                    for k in range(2):
                        nc.tensor.matmul(
                            vsc[:],
                            xt[:, e0 + k * TILE : e0 + (k + 1) * TILE],
                            wvs_sb[:, k * 2 * H : (k + 1) * 2 * H],
                            start=(k == 0),
                            stop=(k == 1),
                        )
                    ex = ex_pool.tile([128, 256], bf16, tag="ex", name=f"ex{tidx}")
                    nc.scalar.activation(ex[:], vsc[:, H : 2 * H], AF.Exp)
                    vs = vs_ring[tidx % NRING]
                    nc.vector.tensor_copy(
                        vs[:].rearrange("p (b c) -> p b c", c=129)[:, :, 0:128],
                        vsc[:, 0:H].rearrange("p (b c) -> p b c", c=128),
                    )
                    pending.append((sl, fi, la, ex, vs))
                    while len(pending) > 2:
                        emit_pooled(*pending.popleft())
                    tidx += 1
            while pending:
                emit_pooled(*pending.popleft())

        # w1 load — one big blocked DMA on the Scalar HWDGE ring, so its 4MB
        # transfer cannot queue ahead of the edge-loop xt groups on the Sync
        # ring (the scheduler hoists it regardless of trace position).
        nc.scalar.dma_start(
            w1_sb[:].rearrange("p (k c) -> p k c", c=H),
            w1_d[:].rearrange("(k p) c -> p k c", p=128),
        )

        # ---- MLP tail ----------------------------------------------------
        with (
            tc.tile_pool(name="mlpp", bufs=2, space="PSUM") as mp,
            tc.tile_pool(name="mlps", bufs=2) as ms,
        ):
            # memset clears PSUM garbage in the never-written rows (NaN-safe
            # for the qsel contraction); it runs during the final extraction
            # window, off the critical path.
            h1pp = mp.tile([128, H], f32, tag="h1pp")
            nc.vector.memset(h1pp[:], 0.0)
            for j in range(64):
                q = j % 4
                nc.tensor.matmul(
                    h1pp[q * 32 : q * 32 + NG, :],
                    P2[:, j * NG : (j + 1) * NG],
                    w1_sb[:, j * H : (j + 1) * H],
                    start=(j < 4),
                    stop=(j >= 60),
                    tile_position=(0, q * 32),
                    skip_group_check=True,
                )
            h1ps = ms.tile([128, H], bf16, tag="h1ps")
            nc.vector.tensor_copy(h1ps[:], h1pp[:])
            # h1^T halves via one matmul each (stationary = h1ps half,
            # moving = qsel): htp[n, g] = sum_p h1ps[p, m*128+n] qsel[p, g].
            # In this transposed layout b1 is per-partition, so it rides the
            # Silu's ACT bias input — no PE transposes, no staging copies.
            h1t = []
            for m in range(2):
                htp = mp.tile([128, NG], f32, tag="htp", name=f"htp{m}")
                nc.tensor.matmul(
                    htp[:],
                    h1ps[:, m * 128 : (m + 1) * 128],
                    qsel_sb[:],
                    start=True,
                    stop=True,
                )
                ht = ms.tile([128, NG], bf16, tag=f"h1t{m}")
                nc.scalar.activation(
                    ht[:], htp[:], AF.Silu, bias=b1_sb[:, m : m + 1]
                )
                h1t.append(ht)
            osb = ms.tile([128, 2 * NG], f32, tag="osb")
            for m in range(2):
                otp = mp.tile([128, NG], f32, tag="otp", name=f"otp{m}")
                for k in range(2):
                    nc.tensor.matmul(
                        otp[:],
                        w2_sb[:, k * H + m * 128 : k * H + m * 128 + 128],
                        h1t[k][:],
                        start=(k == 0),
                        stop=(k == 1),
                    )
                nc.vector.tensor_scalar_add(
                    osb[:, m * NG : (m + 1) * NG], otp[:], b2_sb[:, m : m + 1]
                )
            nc.sync.dma_start(
                outT_d[:].rearrange("(m p) g -> p m g", m=2),
                osb[:].rearrange("p (m g) -> p m g", m=2),
            )

    return nc


def get_program(slot_tiles: tuple[int, ...]) -> "bass.Bass":
    if slot_tiles not in _PROGRAM_CACHE:
        nc = build_program(slot_tiles)
        # HW-path only (CoreSim snapshots the program before this pass)
        _fix_excess_waits(nc)
        _PROGRAM_CACHE[slot_tiles] = nc
    return _PROGRAM_CACHE[slot_tiles]


# ---------------------------------------------------------------------------
# Host-side sharding / padding


def plan_shards(batch: np.ndarray):
    """Returns (assign [NCORES][NG] graph ids, slot_tiles tuple, sizes)."""
    sizes = np.bincount(batch, minlength=B).astype(np.int64)
    order = np.argsort(-sizes, kind="stable")
    assign = [[] for _ in range(NCORES)]
    for r in range(NG):
        row = order[r * NCORES : (r + 1) * NCORES]
        if r % 2 == 1:
            row = row[::-1]
        for c in range(NCORES):
            assign[c].append(int(row[c]))
    for c in range(NCORES):
        assign[c].sort(key=lambda g: -sizes[g])
    slot_tiles = []
    for j in range(NG):
        mx = max(sizes[assign[c][j]] for c in range(NCORES))
        slot_tiles.append(int(max(1, -(-mx // TILE))))
    # round total tiles up to a GROUP multiple (pad goes to the last slot)
    rem = (-sum(slot_tiles)) % (GROUP // TILE)
    slot_tiles[-1] += rem
    return assign, tuple(slot_tiles), sizes


def make_in_maps(edge_features, batch, seed_vectors, w_q, w_k, w_v, w1, b1, w2, b2):
    edge_features = np.asarray(edge_features, dtype=np.float32)
    batch = np.asarray(batch)
    assign, slot_tiles, sizes = plan_shards(batch)
    TT = sum(slot_tiles)
    EC = TT * TILE

    starts = np.searchsorted(batch, np.arange(B))
    xb = edge_features.astype(BF16)

    # Ws[hin, h*S+s] = sum_d w_k[hin, h*HD+d] * q[s, h, d] / sqrt(HD)
    q = (np.asarray(seed_vectors, np.float32) @ np.asarray(w_q, np.float32)).reshape(
        S, NH, HD
    )
    wk3 = np.asarray(w_k, np.float32).reshape(H, NH, HD)
    Ws = (np.einsum("ihd,shd->ihs", wk3, q) * SCALE).reshape(H, NH * S)
    wvs = np.concatenate([np.asarray(w_v, np.float32), Ws], axis=1)

    shared = {
        "wvs": np.ascontiguousarray(wvs.astype(BF16)),
        "w1": np.ascontiguousarray(np.asarray(w1).astype(BF16)),
        "w2": np.ascontiguousarray(np.asarray(w2).astype(BF16)),
        "b1": np.ascontiguousarray(
            np.asarray(b1, dtype=np.float32).reshape(2, 128).T
        ),
        "b2": np.ascontiguousarray(np.asarray(b2, dtype=np.float32).reshape(H, 1)),
        "qsel": np.ascontiguousarray(
            (np.arange(128)[:, None] % 32 == np.arange(NG)[None, :]).astype(BF16)
        ),
    }

    in_maps = []
    for c in range(NCORES):
        xt = np.zeros((H, EC), dtype=BF16)
        npad = np.zeros(NG, dtype=np.float32)
        off = 0
        for j, g in enumerate(assign[c]):
            n = int(sizes[g])
            xt[:, off : off + n] = xb[starts[g] : starts[g] + n].T
            npad[j] = slot_tiles[j] * TILE - n
            off += slot_tiles[j] * TILE
        # k-interleave: [2, 128, TT, 128] (k, p, t, c) -> [128, TT, 2, 128]
        xti = np.ascontiguousarray(
            xt.reshape(2, 128, TT, TILE).transpose(1, 2, 0, 3).reshape(128, 2 * EC)
        )
        m = dict(shared)
        m["xt"] = xti
        m["npad"] = np.ascontiguousarray(np.broadcast_to(npad, (128, NG)))
        in_maps.append(m)
    return in_maps, assign, slot_tiles


def kernel(
    edge_features,
    edge_coords,
    batch,
    seed_vectors,
    w_q,
    w_k,
    w_v,
    w1,
    b1,
    w2,
    b2,
):
    in_maps, assign, slot_tiles = make_in_maps(
        edge_features, batch, seed_vectors, w_q, w_k, w_v, w1, b1, w2, b2
    )
    nc = get_program(slot_tiles)

    res = run_bass_kernel_spmd(nc, in_maps, core_ids=list(range(NCORES)))
    global LAST_RESULTS
    LAST_RESULTS = res

    out = np.zeros((B, H), dtype=np.float32)
    for c in range(NCORES):
        outT = res.results[c]["outT"]  # [H, NG]
        for j, g in enumerate(assign[c]):
            out[g, :] = outT[:, j]
    return out

